# revision 15
# baseline (speedup 1.0000x reference)
"""BBoxEncoder Trainium2 kernel (v4).

Per ray r, BVH level l (8 levels), the reference gathers an embedding row
f = bbox_emb[history[r, l]] (8 corners x 32 dims), normalizes the ray's 16
sample points into the node's AABB, builds trilinear corner weights
w[p, c] and emits feat[r, l, p, d] = sum_c w[p, c] * f[c, d].

v4 vs the v3 baseline (1.89 ms):
  - gathers: one TIE-accelerated `dma_gather` per tile (0.34 ns/descriptor
    on the Q7) replaces 32 `indirect_dma_start` (63 ns/descriptor) -- the
    baseline was GPSIMD-bound at 98% on gather descriptor generation.
    dma_gather needs int16 indices, so the host dedupes each core's node
    ids (<= 32768 draws -> remap fits int16) and ships a per-core
    sub-table instead of the replicated 64 MB table.
  - corner-product mults on DVE in fp16 2x_1P mode: weights stored as
    duplicated fp16 pairs (w_pair[.., p, 2]) so every AP's innermost dim
    is step-1 -- the stride-0 d-broadcast that forced 1x mode moves to a
    middle dim the mode detector ignores.
  - the 8-corner accumulation runs on the Tensor engine: per 512-col
    ray-level block, 8 accumulating identity matmuls sum the corner
    products in PSUM fp32 (was: 7 more DVE passes).
  - the Scalar engine drains PSUM -> SBUF with an fp16-casting copy.

kernel(**inputs) takes the FULL unsharded inputs and returns the FULL
(32768, 4096) float32 output; rays are sharded across 8 cores, host
precomputes per-(ray,level) [nmin | 1/extent] and upcasts the output.
"""

import os as _os

import numpy as np
from contextlib import ExitStack

import concourse.bass as bass
import concourse.tile as tile
from concourse import bacc, mybir
from concourse.bass import IndirectOffsetOnAxis, ts
from concourse.bass_utils import run_bass_kernel_spmd

F32 = mybir.dt.float32
F16 = mybir.dt.float16
I32 = mybir.dt.int32
I16 = mybir.dt.int16
AL = mybir.AluOpType
AF = mybir.ActivationFunctionType

ENC_DEPTH = 8
N_POINTS = 16
ENC_DIM = 32

N_CORES = 8
N_RAYS = 32768
N_NODES = 65536
EMB_ROWS = 32768  # per-core deduped sub-table rows (padded)

J = int(_os.environ.get("KERNEL_RPP", "2"))  # rays per partition
QRL = 4  # ray-levels per PSUM flight (4 banks)
_MAC = _os.environ.get("KERNEL_MAC", "pe")  # "pe" (v4) | "dve" (v3 fallback)

# Tensors replicated across cores (read by sibling tools).
REPLICATED = {"ident"}

# corner order used on-device: c = bx*4 + by*2 + bz (x-bit major).
# reference order (torch chunk order): 000,100,010,001,101,011,110,111
_REF_CORNERS = [
    (0, 0, 0), (1, 0, 0), (0, 1, 0), (0, 0, 1),
    (1, 0, 1), (0, 1, 1), (1, 1, 0), (1, 1, 1),
]
PERM = [0] * 8
for _i, (_bx, _by, _bz) in enumerate(_REF_CORNERS):
    PERM[_bx * 4 + _by * 2 + _bz] = _i


def _emit(ctx: ExitStack, tc, io, n_shard, n_levels, reps=1):
    """Emit the per-core program. io: dict of DRAM tensor handles."""
    nc = tc.nc
    P = 128
    L = n_levels
    JL = J * L
    rays_per_tile = P * J
    n_tiles = n_shard // rays_per_tile
    PD = N_POINTS * ENC_DIM  # 512
    OUT_W = ENC_DEPTH * PD  # 4096
    n_idx_tile = JL * P  # gather descriptors per tile
    idx_cols = n_idx_tile // 16  # wrapped int16 columns per tile

    inp_d = io["inp"].ap()        # (n_shard, 48) f32
    geo_d = io["geo"].ap()        # (n_shard, L*6) f32  [nmin | inv_ext]
    idx_d = io["idx16"].ap()      # (128, n_tiles*idx_cols) i16, wrapped
    emb_d = io["emb"]             # (EMB_ROWS, 256) f16, deduped+permuted
    out_d = io["out"].ap()        # (n_shard, 4096) f16

    ld = ctx.enter_context(tc.tile_pool(name="ld", bufs=4))
    gat = ctx.enter_context(tc.tile_pool(name="gat", bufs=6))
    wrk = ctx.enter_context(tc.tile_pool(name="wrk", bufs=2))
    mac = ctx.enter_context(tc.tile_pool(name="mac", bufs=6))
    acc_p = ctx.enter_context(tc.tile_pool(name="acc", bufs=3))
    const_p = ctx.enter_context(tc.tile_pool(name="const", bufs=1))
    psum_p = ctx.enter_context(tc.tile_pool(name="ps", bufs=2, space="PSUM"))

    # two identical identity tiles: alternating the matmul stationary lets
    # the PE pull the next LDWEIGHTS into the other weight buffer while the
    # current matmul streams (same-tile reloads serialize).
    ident_t = const_p.tile([P, P], F16, tag="ident")
    nc.sync.dma_start(ident_t[:], io["ident"].ap())
    ident2_t = const_p.tile([P, P], F16, tag="ident2")
    nc.sync.dma_start(ident2_t[:], io["ident"].ap())
    idents = [ident_t, ident2_t]
    idx_t = const_p.tile([P, n_tiles * idx_cols], I16, tag="idx")
    nc.sync.dma_start(idx_t[:], idx_d)

    for i in [i for _ in range(reps) for i in range(n_tiles)]:
        r0 = i * rays_per_tile
        inp_t = ld.tile([P, J * 48], F32, tag="inp")
        nc.sync.dma_start(inp_t[:], inp_d[r0:r0 + rays_per_tile, :])
        geo_t = ld.tile([P, J * L * 6], F32, tag="geo")
        nc.sync.dma_start(geo_t[:], geo_d[r0:r0 + rays_per_tile, :])

        # gather: the tile's JL*128 embedding rows via dma_gather
        # (out[q, g, :] = emb[idx[g*128+q], :], g = j*L + l)
        f_t = gat.tile([P, JL * 256], F16, tag="f")
        # >1024 idxs per dma_gather overflows the per-lane SWDGE descriptor
        # ring on HW (NRT_EXEC_UNIT_UNRECOVERABLE); 1024 -> 65 descs/lane.
        gch = int(_os.environ.get("KERNEL_GCHUNK", "512"))
        for gi, g0 in enumerate(range(0, n_idx_tile, gch)):
            gn = min(gch, n_idx_tile - g0)
            rl_a, rl_b = g0 // P, (g0 + gn) // P
            c_a = i * idx_cols + g0 // 16
            nc.gpsimd.dma_gather(
                out_ap=f_t[:, rl_a * 256:rl_b * 256].rearrange(
                    "q (g e) -> q g e", e=256),
                in_ap=emb_d.ap(),
                idxs_ap=idx_t[:, c_a:c_a + gn // 16],
                num_idxs=gn,
                num_idxs_reg=gn,
                elem_size=256,
                queue_num=gi % 2,
            )

        g_v = geo_t[:].rearrange("q (j l e) -> q j l e", j=J, e=6)
        g_jl = geo_t[:].rearrange("q (jl e) -> q jl e", e=6)  # (P, JL, 6)

        # x[q, jl, p, e] = clip((inp[q, j, p, e] - nmin[q, jl, e]) * inv)
        x_t = wrk.tile([P, JL * 48], F32, tag="x")
        x_v = x_t[:].rearrange(
            "q (j l p e) -> q j l p e", j=J, p=N_POINTS, e=3)
        x_jl = x_t[:].rearrange(
            "q (jl p e) -> q jl p e", p=N_POINTS, e=3)
        # coordinate math on GPSIMD (queues freed it); clip via ScalarE relus
        for j in range(J):
            inp_b = (
                inp_t[:, j * 48:(j + 1) * 48]
                .rearrange("q (p e) -> q p e", e=3)
                .unsqueeze(1).to_broadcast([P, L, N_POINTS, 3])
            )
            nmin_b = (g_v[:, j, :, 0:3].unsqueeze(2)
                      .to_broadcast([P, L, N_POINTS, 3]))
            nc.gpsimd.tensor_tensor(
                out=x_v[:, j], in0=inp_b, in1=nmin_b, op=AL.subtract)
        inv_b = (g_jl[:, :, 3:6].unsqueeze(2)
                 .to_broadcast([P, JL, N_POINTS, 3]))
        nc.gpsimd.tensor_tensor(out=x_jl, in0=x_jl, in1=inv_b, op=AL.mult)

        # ft[q, axis, jl, s, p]: s=0 -> 1-t, s=1 -> t   (fp16), t=clip(x,0,1)
        # computed on ScalarE: t' = relu(x); ft0 = relu(1-t') = 1-clip(x);
        # ft1 = 1-ft0 = clip(x)
        ft_t = wrk.tile([P, 3 * JL * 2 * N_POINTS], F16, tag="ft")
        ft_v = ft_t[:].rearrange(
            "q (a jl s p) -> q a jl s p", a=3, s=2, p=N_POINTS
        )
        x_w = x_jl.transpose([0, 3, 1, 2])          # (P, 3, JL, 16) view
        ft0 = ft_v[:, :, :, 0, :].transpose([0, 2, 3, 1])
        ft1 = ft_v[:, :, :, 1, :].transpose([0, 2, 3, 1])
        nc.scalar.activation(
            out=ft1, in_=x_w.transpose([0, 2, 3, 1]), func=AF.Relu)
        nc.scalar.activation(
            out=ft0, in_=ft1, func=AF.Relu, scale=-1.0, bias=1.0)
        nc.scalar.activation(
            out=ft1, in_=ft0, func=AF.Copy, scale=-1.0, bias=1.0)

        # wxy[q, jl, bx, by, p] then w[q, jl, bx, by, bz, p] on GPSIMD (fp16)
        wxy_t = wrk.tile([P, JL * 4 * N_POINTS], F16, tag="wxy")
        wxy_v = wxy_t[:].rearrange(
            "q (jl x y p) -> q jl x y p", x=2, y=2, p=N_POINTS
        )
        for bx in range(2):
            nc.gpsimd.tensor_tensor(
                out=wxy_v[:, :, bx],
                in0=ft_v[:, 0, :, bx, :].unsqueeze(2)
                    .to_broadcast([P, JL, 2, N_POINTS]),
                in1=ft_v[:, 1],
                op=AL.mult,
            )
        w_t = wrk.tile([P, JL * 8 * N_POINTS], F16, tag="w")
        w_v = w_t[:].rearrange(
            "q (jl x y z p) -> q jl x y z p", x=2, y=2, z=2, p=N_POINTS
        )
        for bx in range(2):
            for by in range(2):
                nc.gpsimd.tensor_tensor(
                    out=w_v[:, :, bx, by],
                    in0=wxy_v[:, :, bx, by, :].unsqueeze(2)
                        .to_broadcast([P, JL, 2, N_POINTS]),
                    in1=ft_v[:, 2],
                    op=AL.mult,
                )

        acc_t = acc_p.tile([P, J * OUT_W], F16, tag="acc")

        if _MAC == "pe":
            # w_pair[q, jl, c, p, 2]: each weight duplicated into an fp16
            # pair so the MAC mult reads it with a step-1 innermost dim.
            # (walrus caps compute APs at 3 free dims, so the copy uses
            # merged views and the mults run one per (corner, ray-level).)
            wp_t = wrk.tile([P, JL * 8 * N_POINTS * 2], F16, tag="wp")
            wp_2 = wp_t[:].rearrange("q (wi two) -> q wi two", two=2)
            w_src = w_t[:].unsqueeze(2).to_broadcast(
                [P, JL * 8 * N_POINTS, 2])
            nc.scalar.copy(out=wp_2, in_=w_src)
            wp_v = wp_t[:].rearrange(
                "q (jl c p two) -> q jl c p two", c=8, p=N_POINTS, two=2)

            f_5 = f_t[:].rearrange(
                "q (jl c dh two) -> q jl c dh two", c=8, dh=16, two=2)

            n_q = (JL + QRL - 1) // QRL
            for q0 in range(n_q):
                rl0 = q0 * QRL
                rls = min(QRL, JL - rl0)
                ps_t = psum_p.tile([P, QRL * PD], F32, tag="ps")
                for c in range(8):
                    tmp_t = mac.tile([P, QRL * PD], F16, tag="tmp")
                    for b in range(rls):
                        rl = rl0 + b
                        out_v = tmp_t[:, b * PD:(b + 1) * PD].rearrange(
                            "q (p dh two) -> q p dh two", dh=16, two=2)
                        in0 = (wp_v[:, rl, c].unsqueeze(2)
                               .to_broadcast([P, N_POINTS, 16, 2]))
                        in1 = (f_5[:, rl, c].unsqueeze(1)
                               .to_broadcast([P, N_POINTS, 16, 2]))
                        nc.vector.tensor_tensor(
                            out=out_v, in0=in0, in1=in1, op=AL.mult)
                    for b in range(rls):
                        nc.tensor.matmul(
                            out=ps_t[:, b * PD:(b + 1) * PD],
                            lhsT=idents[(c * rls + b) % 2][:],
                            rhs=tmp_t[:, b * PD:(b + 1) * PD],
                            start=(c == 0), stop=(c == 7),
                        )
                # drain PSUM -> acc (fp32 -> fp16) on the Scalar engine
                if L == ENC_DEPTH:
                    nc.scalar.copy(
                        out=acc_t[:, rl0 * PD:(rl0 + rls) * PD],
                        in_=ps_t[:, :rls * PD],
                    )
                else:
                    for b in range(rls):
                        jl = rl0 + b
                        j, l = jl // L, jl % L
                        off = j * OUT_W + l * PD
                        nc.scalar.copy(
                            out=acc_t[:, off:off + PD],
                            in_=ps_t[:, b * PD:(b + 1) * PD],
                        )
        else:
            # v3 fallback: all-DVE MAC (broadcast mults at 1x + adds)
            w_c = w_t[:].rearrange("q (jl c p) -> q jl c p", c=8, p=N_POINTS)
            f_c = f_t[:].rearrange("q (jl c d) -> q jl c d", c=8, d=ENC_DIM)
            acc_v = acc_t[:].rearrange(
                "q (j lw) -> q j lw", j=J)[:, :, : L * PD].rearrange(
                "q j (l p d) -> q (j l) p d", p=N_POINTS, d=ENC_DIM) \
                if L < ENC_DEPTH else \
                acc_t[:].rearrange(
                    "q (jl p d) -> q jl p d", p=N_POINTS, d=ENC_DIM)
            tmp_t = mac.tile([P, JL * PD], F16, tag="tmp3")
            tmp_v = tmp_t[:].rearrange(
                "q (jl p d) -> q jl p d", p=N_POINTS, d=ENC_DIM)
            for c in range(8):
                dst = acc_v if c == 0 else tmp_v
                w_b = (w_c[:, :, c, :].unsqueeze(3)
                       .to_broadcast([P, JL, N_POINTS, ENC_DIM]))
                f_b = (f_c[:, :, c, :].unsqueeze(2)
                       .to_broadcast([P, JL, N_POINTS, ENC_DIM]))
                nc.vector.tensor_tensor(out=dst, in0=w_b, in1=f_b, op=AL.mult)
                if c > 0:
                    nc.vector.tensor_tensor(
                        out=acc_v, in0=acc_v, in1=tmp_v, op=AL.add)

        if L < ENC_DEPTH:
            av = acc_t[:].rearrange("q (j lw) -> q j lw", j=J)
            nc.gpsimd.memset(av[:, :, L * PD:], 0.0)

        nc.sync.dma_start(out_d[r0:r0 + rays_per_tile, :], acc_t[:])


def build_program(n_shard, n_levels, reps=1):
    nc = bacc.Bacc(
        "TRN2", target_bir_lowering=False, debug=False, enable_asserts=False,
        num_swdge_queues=2,
    )
    L = n_levels
    JL = J * L
    n_tiles = n_shard // (128 * J)
    idx_cols_total = n_tiles * (JL * 128 // 16)
    io = {
        "inp": nc.dram_tensor("inp", [n_shard, 48], F32, kind="ExternalInput"),
        "geo": nc.dram_tensor("geo", [n_shard, n_levels * 6], F32,
                              kind="ExternalInput"),
        "idx16": nc.dram_tensor("idx16", [128, idx_cols_total], I16,
                                kind="ExternalInput"),
        "emb": nc.dram_tensor("emb", [EMB_ROWS, 256], F16,
                              kind="ExternalInput"),
        "ident": nc.dram_tensor("ident", [128, 128], F16,
                                kind="ExternalInput"),
        "out": nc.dram_tensor(
            "out", [n_shard, ENC_DEPTH * N_POINTS * ENC_DIM], F16,
            kind="ExternalOutput",
        ),
    }
    with tile.TileContext(nc) as tc, ExitStack() as ctx:
        _emit(ctx, tc, io, n_shard, n_levels, reps=reps)
    nc.compile()
    return nc


_CACHE = {}


def _get_program(n_shard, n_levels):
    key = (n_shard, n_levels)
    if key not in _CACHE:
        _CACHE[key] = build_program(n_shard, n_levels)
    return _CACHE[key]


def make_in_maps(inp, history, bbox_emb, nodes_min, nodes_max, n_cores=N_CORES,
                 n_levels=ENC_DEPTH):
    """Host-side marshalling: shard rays, permute emb corners, gather geo,
    dedupe per-core node ids into int16 + per-core sub-table."""
    n_rays = inp.shape[0]
    shard = n_rays // n_cores
    L = n_levels
    JL = J * L
    n_tiles = shard // (128 * J)
    inp_f = np.ascontiguousarray(
        inp.reshape(n_rays, 48).astype(np.float32, copy=False)
    )
    hist8 = np.ascontiguousarray(
        history[:, :ENC_DEPTH].astype(np.int32, copy=False))
    nmin = nodes_min.astype(np.float32, copy=False)
    ext = (nodes_max - nodes_min).astype(np.float32, copy=False)
    ext = np.where(ext == 0, np.float32(0.5), ext)
    geo_tab = np.concatenate([nmin, (1.0 / ext).astype(np.float32)], axis=1)
    geo = np.ascontiguousarray(
        geo_tab[hist8[:, :L]].reshape(n_rays, L * 6))
    n_nodes = nodes_min.shape[0]
    emb_p = np.ascontiguousarray(
        bbox_emb.astype(np.float32, copy=False)
        .reshape(n_nodes, 8, ENC_DIM)[:, PERM, :]
        .reshape(n_nodes, 8 * ENC_DIM)
        .astype(np.float16)
    )
    ident = np.eye(128, dtype=np.float16)
    in_maps = []
    for c in range(n_cores):
        sl = slice(c * shard, (c + 1) * shard)
        h = hist8[sl, :L]                      # (shard, L)
        uniq, inv = np.unique(h, return_inverse=True)
        assert uniq.size <= EMB_ROWS
        emb_local = np.zeros((EMB_ROWS, 256), np.float16)
        emb_local[:uniq.size] = emb_p[uniq]
        h16 = inv.reshape(shard, L).astype(np.int16)
        # gather order: k = ((tile*JL) + j*L + l)*128 + q, ray = q*J + j
        hs = h16.reshape(n_tiles, 128, J, L).transpose(0, 2, 3, 1)
        flat = np.ascontiguousarray(hs).reshape(-1)   # (n_tiles*JL*128,)
        wrapped = np.ascontiguousarray(flat.reshape(-1, 16).T)  # (16, S)
        idx16 = np.ascontiguousarray(
            np.tile(wrapped, (8, 1)))          # (128, S) replicated per Q7
        in_maps.append({
            "inp": inp_f[sl],
            "geo": geo[sl],
            "idx16": idx16,
            "emb": emb_local,
            "ident": ident,
        })
    return in_maps, shard, n_nodes


def kernel(inp, history, depth, bbox_emb, nodes_min, nodes_max):
    inp = np.asarray(inp)
    history = np.asarray(history)
    depth = np.asarray(depth)
    bbox_emb = np.asarray(bbox_emb)
    nodes_min = np.asarray(nodes_min)
    nodes_max = np.asarray(nodes_max)

    n_rays = inp.shape[0]
    n_levels = int(min(int(depth.max()), ENC_DEPTH)) if depth.size else 0
    if n_levels <= 0:
        return np.zeros((n_rays, ENC_DEPTH * N_POINTS * ENC_DIM), np.float32)
    in_maps, shard, n_nodes = make_in_maps(
        inp, history, bbox_emb, nodes_min, nodes_max, n_levels=n_levels
    )

    nc = _get_program(shard, n_levels)
    res = run_bass_kernel_spmd(nc, in_maps, core_ids=list(range(N_CORES)))
    out = np.concatenate(
        [r["out"].astype(np.float32) for r in res.results], axis=0)
    return out


# revision 17
# speedup vs baseline: 1.5659x; 1.5659x over previous
"""BBoxEncoder Trainium2 kernel (v4).

Per ray r, BVH level l (8 levels), the reference gathers an embedding row
f = bbox_emb[history[r, l]] (8 corners x 32 dims), normalizes the ray's 16
sample points into the node's AABB, builds trilinear corner weights
w[p, c] and emits feat[r, l, p, d] = sum_c w[p, c] * f[c, d].

v4 vs the v3 baseline (1.89 ms):
  - gathers: one TIE-accelerated `dma_gather` per tile (0.34 ns/descriptor
    on the Q7) replaces 32 `indirect_dma_start` (63 ns/descriptor) -- the
    baseline was GPSIMD-bound at 98% on gather descriptor generation.
    dma_gather needs int16 indices, so the host dedupes each core's node
    ids (<= 32768 draws -> remap fits int16) and ships a per-core
    sub-table instead of the replicated 64 MB table.
  - corner-product mults on DVE in fp16 2x_1P mode: weights stored as
    duplicated fp16 pairs (w_pair[.., p, 2]) so every AP's innermost dim
    is step-1 -- the stride-0 d-broadcast that forced 1x mode moves to a
    middle dim the mode detector ignores.
  - the 8-corner accumulation runs on the Tensor engine: per 512-col
    ray-level block, 8 accumulating identity matmuls sum the corner
    products in PSUM fp32 (was: 7 more DVE passes).
  - the Scalar engine drains PSUM -> SBUF with an fp16-casting copy.

kernel(**inputs) takes the FULL unsharded inputs and returns the FULL
(32768, 4096) float32 output; rays are sharded across 8 cores, host
precomputes per-(ray,level) [nmin | 1/extent] and upcasts the output.
"""

import os as _os

import numpy as np
from contextlib import ExitStack

import concourse.bass as bass
import concourse.tile as tile
from concourse import bacc, mybir
from concourse.bass import IndirectOffsetOnAxis, ts
from concourse.bass_utils import run_bass_kernel_spmd

F32 = mybir.dt.float32
F16 = mybir.dt.float16
I32 = mybir.dt.int32
I16 = mybir.dt.int16
AL = mybir.AluOpType
AF = mybir.ActivationFunctionType

ENC_DEPTH = 8
N_POINTS = 16
ENC_DIM = 32

N_CORES = 8
N_RAYS = 32768
N_NODES = 65536
EMB_ROWS = 32768  # per-core deduped sub-table rows (padded)

J = int(_os.environ.get("KERNEL_RPP", "2"))  # rays per partition
QRL = 4  # ray-levels per PSUM flight (4 banks)
_MAC = _os.environ.get("KERNEL_MAC", "pe")  # "pe" (v4) | "dve" (v3 fallback)

# Tensors replicated across cores (read by sibling tools).
REPLICATED = {"ident"}

# corner order used on-device: c = bx*4 + by*2 + bz (x-bit major).
# reference order (torch chunk order): 000,100,010,001,101,011,110,111
_REF_CORNERS = [
    (0, 0, 0), (1, 0, 0), (0, 1, 0), (0, 0, 1),
    (1, 0, 1), (0, 1, 1), (1, 1, 0), (1, 1, 1),
]
PERM = [0] * 8
for _i, (_bx, _by, _bz) in enumerate(_REF_CORNERS):
    PERM[_bx * 4 + _by * 2 + _bz] = _i


def _emit(ctx: ExitStack, tc, io, n_shard, n_levels, reps=1):
    """Emit the per-core program. io: dict of DRAM tensor handles."""
    nc = tc.nc
    P = 128
    L = n_levels
    JL = J * L
    rays_per_tile = P * J
    n_tiles = n_shard // rays_per_tile
    PD = N_POINTS * ENC_DIM  # 512
    OUT_W = ENC_DEPTH * PD  # 4096
    n_idx_tile = JL * P  # gather descriptors per tile
    idx_cols = n_idx_tile // 16  # wrapped int16 columns per tile

    inp_d = io["inp"].ap()        # (n_shard, 48) f32
    geo_d = io["geo"].ap()        # (n_shard, L*6) f32  [nmin | inv_ext]
    idx_d = io["idx16"].ap()      # (128, n_tiles*idx_cols) i16, wrapped
    emb_d = io["emb"]             # (EMB_ROWS, 256) f16, deduped+permuted
    out_d = io["out"].ap()        # (n_shard, 4096) f16

    ld = ctx.enter_context(tc.tile_pool(name="ld", bufs=4))
    gat = ctx.enter_context(tc.tile_pool(name="gat", bufs=6))
    wrk = ctx.enter_context(tc.tile_pool(name="wrk", bufs=2))
    mac = ctx.enter_context(tc.tile_pool(name="mac", bufs=6))
    acc_p = ctx.enter_context(tc.tile_pool(name="acc", bufs=3))
    const_p = ctx.enter_context(tc.tile_pool(name="const", bufs=1))
    psum_p = ctx.enter_context(tc.tile_pool(name="ps", bufs=2, space="PSUM"))

    # two identical identity tiles: alternating the matmul stationary lets
    # the PE pull the next LDWEIGHTS into the other weight buffer while the
    # current matmul streams (same-tile reloads serialize).
    ident_t = const_p.tile([P, P], F16, tag="ident")
    nc.sync.dma_start(ident_t[:], io["ident"].ap())
    ident2_t = const_p.tile([P, P], F16, tag="ident2")
    nc.sync.dma_start(ident2_t[:], io["ident"].ap())
    idents = [ident_t, ident2_t]
    idx_t = const_p.tile([P, n_tiles * idx_cols], I16, tag="idx")
    nc.sync.dma_start(idx_t[:], idx_d)

    for i in [i for _ in range(reps) for i in range(n_tiles)]:
        r0 = i * rays_per_tile
        inp_t = ld.tile([P, J * 48], F32, tag="inp")
        nc.sync.dma_start(inp_t[:], inp_d[r0:r0 + rays_per_tile, :])
        geo_t = ld.tile([P, J * L * 6], F32, tag="geo")
        nc.sync.dma_start(geo_t[:], geo_d[r0:r0 + rays_per_tile, :])

        # gather: the tile's JL*128 embedding rows via dma_gather
        # (out[q, g, :] = emb[idx[g*128+q], :], g = j*L + l)
        f_t = gat.tile([P, JL * 256], F16, tag="f")
        # >1024 idxs per dma_gather overflows the per-lane SWDGE descriptor
        # ring on HW (NRT_EXEC_UNIT_UNRECOVERABLE); 1024 -> 65 descs/lane.
        gch = int(_os.environ.get("KERNEL_GCHUNK", "512"))
        for gi, g0 in enumerate(range(0, n_idx_tile, gch)):
            gn = min(gch, n_idx_tile - g0)
            rl_a, rl_b = g0 // P, (g0 + gn) // P
            c_a = i * idx_cols + g0 // 16
            nc.gpsimd.dma_gather(
                out_ap=f_t[:, rl_a * 256:rl_b * 256].rearrange(
                    "q (g e) -> q g e", e=256),
                in_ap=emb_d.ap(),
                idxs_ap=idx_t[:, c_a:c_a + gn // 16],
                num_idxs=gn,
                num_idxs_reg=gn,
                elem_size=256,
                queue_num=gi % 2,
            )

        g_v = geo_t[:].rearrange("q (j l e) -> q j l e", j=J, e=6)
        g_jl = geo_t[:].rearrange("q (jl e) -> q jl e", e=6)  # (P, JL, 6)

        # x[q, jl, p, e] = clip((inp[q, j, p, e] - nmin[q, jl, e]) * inv)
        x_t = wrk.tile([P, JL * 48], F32, tag="x")
        x_v = x_t[:].rearrange(
            "q (j l p e) -> q j l p e", j=J, p=N_POINTS, e=3)
        x_jl = x_t[:].rearrange(
            "q (jl p e) -> q jl p e", p=N_POINTS, e=3)
        # coordinate math on DVE; clip folded into the ScalarE relu chain
        for j in range(J):
            inp_b = (
                inp_t[:, j * 48:(j + 1) * 48]
                .rearrange("q (p e) -> q p e", e=3)
                .unsqueeze(1).to_broadcast([P, L, N_POINTS, 3])
            )
            nmin_b = (g_v[:, j, :, 0:3].unsqueeze(2)
                      .to_broadcast([P, L, N_POINTS, 3]))
            nc.vector.tensor_tensor(
                out=x_v[:, j], in0=inp_b, in1=nmin_b, op=AL.subtract)
        inv_b = (g_jl[:, :, 3:6].unsqueeze(2)
                 .to_broadcast([P, JL, N_POINTS, 3]))
        nc.vector.tensor_tensor(out=x_jl, in0=x_jl, in1=inv_b, op=AL.mult)

        # ft[q, axis, jl, s, p]: s=0 -> 1-t, s=1 -> t   (fp16), t=clip(x,0,1)
        # computed on ScalarE: t' = relu(x); ft0 = relu(1-t') = 1-clip(x);
        # ft1 = 1-ft0 = clip(x)
        ft_t = wrk.tile([P, 3 * JL * 2 * N_POINTS], F16, tag="ft")
        ft_v = ft_t[:].rearrange(
            "q (a jl s p) -> q a jl s p", a=3, s=2, p=N_POINTS
        )
        x_w = x_jl.transpose([0, 3, 1, 2])          # (P, 3, JL, 16) view
        ft0 = ft_v[:, :, :, 0, :].transpose([0, 2, 3, 1])
        ft1 = ft_v[:, :, :, 1, :].transpose([0, 2, 3, 1])
        nc.scalar.activation(
            out=ft1, in_=x_w.transpose([0, 2, 3, 1]), func=AF.Relu)
        nc.scalar.activation(
            out=ft0, in_=ft1, func=AF.Relu, scale=-1.0, bias=1.0)
        nc.scalar.activation(
            out=ft1, in_=ft0, func=AF.Copy, scale=-1.0, bias=1.0)

        # wxy[q, jl, bx, by, p] then w[q, jl, bx, by, bz, p]   (fp16)
        wxy_t = wrk.tile([P, JL * 4 * N_POINTS], F16, tag="wxy")
        wxy_v = wxy_t[:].rearrange(
            "q (jl x y p) -> q jl x y p", x=2, y=2, p=N_POINTS
        )
        for bx in range(2):
            nc.vector.tensor_tensor(
                out=wxy_v[:, :, bx],
                in0=ft_v[:, 0, :, bx, :].unsqueeze(2)
                    .to_broadcast([P, JL, 2, N_POINTS]),
                in1=ft_v[:, 1],
                op=AL.mult,
            )
        w_t = wrk.tile([P, JL * 8 * N_POINTS], F16, tag="w")
        w_v = w_t[:].rearrange(
            "q (jl x y z p) -> q jl x y z p", x=2, y=2, z=2, p=N_POINTS
        )
        for bx in range(2):
            for by in range(2):
                nc.vector.tensor_tensor(
                    out=w_v[:, :, bx, by],
                    in0=wxy_v[:, :, bx, by, :].unsqueeze(2)
                        .to_broadcast([P, JL, 2, N_POINTS]),
                    in1=ft_v[:, 2],
                    op=AL.mult,
                )

        acc_t = acc_p.tile([P, J * OUT_W], F16, tag="acc")

        if _MAC == "pe":
            # w_pair[q, jl, c, p, 2]: each weight duplicated into an fp16
            # pair so the MAC mult reads it with a step-1 innermost dim.
            # (walrus caps compute APs at 3 free dims, so the copy uses
            # merged views and the mults run one per (corner, ray-level).)
            wp_t = wrk.tile([P, JL * 8 * N_POINTS * 2], F16, tag="wp")
            wp_2 = wp_t[:].rearrange("q (wi two) -> q wi two", two=2)
            w_src = w_t[:].unsqueeze(2).to_broadcast(
                [P, JL * 8 * N_POINTS, 2])
            nc.scalar.copy(out=wp_2, in_=w_src)
            wp_v = wp_t[:].rearrange(
                "q (jl c p two) -> q jl c p two", c=8, p=N_POINTS, two=2)

            f_5 = f_t[:].rearrange(
                "q (jl c dh two) -> q jl c dh two", c=8, dh=16, two=2)

            n_q = (JL + QRL - 1) // QRL
            for q0 in range(n_q):
                rl0 = q0 * QRL
                rls = min(QRL, JL - rl0)
                ps_t = psum_p.tile([P, QRL * PD], F32, tag="ps")
                for c in range(8):
                    tmp_t = mac.tile([P, QRL * PD], F16, tag="tmp")
                    for b in range(rls):
                        rl = rl0 + b
                        out_v = tmp_t[:, b * PD:(b + 1) * PD].rearrange(
                            "q (p dh two) -> q p dh two", dh=16, two=2)
                        in0 = (wp_v[:, rl, c].unsqueeze(2)
                               .to_broadcast([P, N_POINTS, 16, 2]))
                        in1 = (f_5[:, rl, c].unsqueeze(1)
                               .to_broadcast([P, N_POINTS, 16, 2]))
                        nc.vector.tensor_tensor(
                            out=out_v, in0=in0, in1=in1, op=AL.mult)
                    for b in range(rls):
                        nc.tensor.matmul(
                            out=ps_t[:, b * PD:(b + 1) * PD],
                            lhsT=idents[(c * rls + b) % 2][:],
                            rhs=tmp_t[:, b * PD:(b + 1) * PD],
                            start=(c == 0), stop=(c == 7),
                        )
                # drain PSUM -> acc (fp32 -> fp16) on the Scalar engine
                if L == ENC_DEPTH:
                    nc.scalar.copy(
                        out=acc_t[:, rl0 * PD:(rl0 + rls) * PD],
                        in_=ps_t[:, :rls * PD],
                    )
                else:
                    for b in range(rls):
                        jl = rl0 + b
                        j, l = jl // L, jl % L
                        off = j * OUT_W + l * PD
                        nc.scalar.copy(
                            out=acc_t[:, off:off + PD],
                            in_=ps_t[:, b * PD:(b + 1) * PD],
                        )
        else:
            # v3 fallback: all-DVE MAC (broadcast mults at 1x + adds)
            w_c = w_t[:].rearrange("q (jl c p) -> q jl c p", c=8, p=N_POINTS)
            f_c = f_t[:].rearrange("q (jl c d) -> q jl c d", c=8, d=ENC_DIM)
            acc_v = acc_t[:].rearrange(
                "q (j lw) -> q j lw", j=J)[:, :, : L * PD].rearrange(
                "q j (l p d) -> q (j l) p d", p=N_POINTS, d=ENC_DIM) \
                if L < ENC_DEPTH else \
                acc_t[:].rearrange(
                    "q (jl p d) -> q jl p d", p=N_POINTS, d=ENC_DIM)
            tmp_t = mac.tile([P, JL * PD], F16, tag="tmp3")
            tmp_v = tmp_t[:].rearrange(
                "q (jl p d) -> q jl p d", p=N_POINTS, d=ENC_DIM)
            for c in range(8):
                dst = acc_v if c == 0 else tmp_v
                w_b = (w_c[:, :, c, :].unsqueeze(3)
                       .to_broadcast([P, JL, N_POINTS, ENC_DIM]))
                f_b = (f_c[:, :, c, :].unsqueeze(2)
                       .to_broadcast([P, JL, N_POINTS, ENC_DIM]))
                nc.vector.tensor_tensor(out=dst, in0=w_b, in1=f_b, op=AL.mult)
                if c > 0:
                    nc.vector.tensor_tensor(
                        out=acc_v, in0=acc_v, in1=tmp_v, op=AL.add)

        if L < ENC_DEPTH:
            av = acc_t[:].rearrange("q (j lw) -> q j lw", j=J)
            nc.gpsimd.memset(av[:, :, L * PD:], 0.0)

        nc.sync.dma_start(out_d[r0:r0 + rays_per_tile, :], acc_t[:])


def build_program(n_shard, n_levels, reps=1):
    nc = bacc.Bacc(
        "TRN2", target_bir_lowering=False, debug=False, enable_asserts=False,
        num_swdge_queues=2,
    )
    L = n_levels
    JL = J * L
    n_tiles = n_shard // (128 * J)
    idx_cols_total = n_tiles * (JL * 128 // 16)
    io = {
        "inp": nc.dram_tensor("inp", [n_shard, 48], F32, kind="ExternalInput"),
        "geo": nc.dram_tensor("geo", [n_shard, n_levels * 6], F32,
                              kind="ExternalInput"),
        "idx16": nc.dram_tensor("idx16", [128, idx_cols_total], I16,
                                kind="ExternalInput"),
        "emb": nc.dram_tensor("emb", [EMB_ROWS, 256], F16,
                              kind="ExternalInput"),
        "ident": nc.dram_tensor("ident", [128, 128], F16,
                                kind="ExternalInput"),
        "out": nc.dram_tensor(
            "out", [n_shard, ENC_DEPTH * N_POINTS * ENC_DIM], F16,
            kind="ExternalOutput",
        ),
    }
    with tile.TileContext(nc) as tc, ExitStack() as ctx:
        _emit(ctx, tc, io, n_shard, n_levels, reps=reps)
    nc.compile()
    return nc


_CACHE = {}


def _get_program(n_shard, n_levels):
    key = (n_shard, n_levels)
    if key not in _CACHE:
        _CACHE[key] = build_program(n_shard, n_levels)
    return _CACHE[key]


def make_in_maps(inp, history, bbox_emb, nodes_min, nodes_max, n_cores=N_CORES,
                 n_levels=ENC_DEPTH):
    """Host-side marshalling: shard rays, permute emb corners, gather geo,
    dedupe per-core node ids into int16 + per-core sub-table."""
    n_rays = inp.shape[0]
    shard = n_rays // n_cores
    L = n_levels
    JL = J * L
    n_tiles = shard // (128 * J)
    inp_f = np.ascontiguousarray(
        inp.reshape(n_rays, 48).astype(np.float32, copy=False)
    )
    hist8 = np.ascontiguousarray(
        history[:, :ENC_DEPTH].astype(np.int32, copy=False))
    nmin = nodes_min.astype(np.float32, copy=False)
    ext = (nodes_max - nodes_min).astype(np.float32, copy=False)
    ext = np.where(ext == 0, np.float32(0.5), ext)
    geo_tab = np.concatenate([nmin, (1.0 / ext).astype(np.float32)], axis=1)
    geo = np.ascontiguousarray(
        geo_tab[hist8[:, :L]].reshape(n_rays, L * 6))
    n_nodes = nodes_min.shape[0]
    emb_p = np.ascontiguousarray(
        bbox_emb.astype(np.float32, copy=False)
        .reshape(n_nodes, 8, ENC_DIM)[:, PERM, :]
        .reshape(n_nodes, 8 * ENC_DIM)
        .astype(np.float16)
    )
    ident = np.eye(128, dtype=np.float16)
    in_maps = []
    for c in range(n_cores):
        sl = slice(c * shard, (c + 1) * shard)
        h = hist8[sl, :L]                      # (shard, L)
        uniq, inv = np.unique(h, return_inverse=True)
        assert uniq.size <= EMB_ROWS
        emb_local = np.zeros((EMB_ROWS, 256), np.float16)
        emb_local[:uniq.size] = emb_p[uniq]
        h16 = inv.reshape(shard, L).astype(np.int16)
        # gather order: k = ((tile*JL) + j*L + l)*128 + q, ray = q*J + j
        hs = h16.reshape(n_tiles, 128, J, L).transpose(0, 2, 3, 1)
        flat = np.ascontiguousarray(hs).reshape(-1)   # (n_tiles*JL*128,)
        wrapped = np.ascontiguousarray(flat.reshape(-1, 16).T)  # (16, S)
        idx16 = np.ascontiguousarray(
            np.tile(wrapped, (8, 1)))          # (128, S) replicated per Q7
        in_maps.append({
            "inp": inp_f[sl],
            "geo": geo[sl],
            "idx16": idx16,
            "emb": emb_local,
            "ident": ident,
        })
    return in_maps, shard, n_nodes


def kernel(inp, history, depth, bbox_emb, nodes_min, nodes_max):
    inp = np.asarray(inp)
    history = np.asarray(history)
    depth = np.asarray(depth)
    bbox_emb = np.asarray(bbox_emb)
    nodes_min = np.asarray(nodes_min)
    nodes_max = np.asarray(nodes_max)

    n_rays = inp.shape[0]
    n_levels = int(min(int(depth.max()), ENC_DEPTH)) if depth.size else 0
    if n_levels <= 0:
        return np.zeros((n_rays, ENC_DEPTH * N_POINTS * ENC_DIM), np.float32)
    in_maps, shard, n_nodes = make_in_maps(
        inp, history, bbox_emb, nodes_min, nodes_max, n_levels=n_levels
    )

    nc = _get_program(shard, n_levels)
    res = run_bass_kernel_spmd(nc, in_maps, core_ids=list(range(N_CORES)))
    out = np.concatenate(
        [r["out"].astype(np.float32) for r in res.results], axis=0)
    return out
